# revision 46
# baseline (speedup 1.0000x reference)
"""Trainium2 Bass kernel for the DRN histogram-binning module.

Math: the reference computes
    T[j,k,l,m] = exp(-W[j,k] * d[l,m]),   d[l,m] = ((m-l)/64)^2 in [0,1)
    Pw[i,j,k,l] = sum_m T[j,k,l,m] x[i,k,m]
    logsum[i,j,l] = sum_k log(Pw)
    out = softmax(logsum + expB, axis=l)

Because |W| <= 1/8 and sum_m x = 1 (per-feature distributions), Pw is a
weighted mean of exp(z) with |z| <= 0.125, so log(Pw) has a fast-converging
moment expansion.  To cubic order (validated: scale-rel err ~7e-6, equal to
the fp32 noise floor of the reference itself):

    log Pw ~= -W*Y1 + W^2/2*(Y2 - Y1^2) - W^3/6*(Y3 - 3 Y1 Y2 + 2 Y1^3)

with moments Y_p[i,k,l] = sum_m d[l,m]^p x[i,k,m].  Everything reduces to:
  phase A: Y_p = x @ d^p        (per-batch 64x128x192 matmuls)
  phase B: elementwise products Y1^2, Y1*Y2, Y1^3
  phase C: logsum[i,j,l] = sum_g C_g[j,k] (k-contraction) Z_g[i,k,l]
           with C_g = scaled powers of W  (6 groups, K=64 matmuls)
  epilogue: + expB, exp, per-row normalize.

No transcendentals ever touch a large tensor (only one exp on the output-
sized tile).  Sharding: data-parallel over batch, 32 rows per core; batch is
split per-core as i = ih*16 + t with ih in {0,1} mapped to SBUF partition
halves so every elementwise op runs on all 128 partitions.

Host-side work is layout only (shard/transpose/replicate); all arithmetic
that depends on input VALUES (powers of W, expB, moments, ...) runs on
device.  d^p matrices are input-independent compile-time constants.
"""

from contextlib import ExitStack

import numpy as np

import bass_rust
import concourse.bass as bass
import concourse.tile as tile
from concourse import mybir
from concourse.bass_utils import run_bass_kernel_spmd

NCORES = 8
B = 256
BL = B // NCORES          # 32 batch rows per core
TH = BL // 2              # 16 = per-partition-half batch rows
F_IN = 64
F_OUT = 64
QL = 64                   # in bins (m)
QU = 64                   # out bins (l)
DT = mybir.dt.float32

# dstack column scales beta_p chosen so the 6 contraction groups need only
# 4 distinct stored coefficient blocks:
#   blocks: [-W, -W^2/2, -W^3/2, -W^3/3]
#   stored moments: Y1' = Y1, Y2' = -Y2, Y3' = Y3/3
_BETA = (1.0, -1.0, 1.0 / 3.0)
USE_TILE_POS = False

_CACHE: dict = {}


def _split_waits(nc, max_waits=1):
    """The walrus build in this container supports only one sync-wait command
    per instruction; Tile emits several.  Hoist extras onto standalone
    EventSemaphore carrier instructions on the same engine, preserving
    program order (exactly what raw-bass wait_ge emits)."""
    for fn in nc.m.functions:
        for blk in fn.blocks:
            out = []
            changed = False
            for ins in blk.instructions:
                si = getattr(ins, "sync_info", None)
                waits = list(si.on_wait) if si is not None else []
                if len(waits) > max_waits:
                    changed = True
                    for w in waits[:-max_waits]:
                        evt = mybir.InstEventSemaphore(
                            name=nc.get_next_instruction_name(), ins=[], outs=[]
                        )
                        evt.engine = ins.engine
                        evt.sync_info = bass_rust.SyncInfo(on_wait=[w], on_update=[])
                        out.append(evt)
                    ins.sync_info = bass_rust.SyncInfo(
                        on_wait=waits[-max_waits:], on_update=list(si.on_update)
                    )
                out.append(ins)
            if changed:
                blk.instructions = out


def _host_consts():
    s0 = np.arange(QL, dtype=np.float64) / QL
    s1 = np.arange(QU, dtype=np.float64) / QU
    d = (s0[None, :] - s1[:, None]) ** 2          # (l, m)
    dT = d.T                                       # (m, l)
    ds = np.concatenate(
        [_BETA[p - 1] * dT ** p for p in (1, 2, 3)] + [np.zeros((QL, QU))], axis=1
    )
    # padded with a zero block to 256 cols: float32r matmuls hit full rate
    # only when the moving free dim is >= 256
    dstack = np.ascontiguousarray(ds, dtype=np.float32)          # (64, 256)
    smat = np.broadcast_to((np.arange(QU) / QU).astype(np.float32), (128, QU))
    smat = np.ascontiguousarray(smat)                            # (128, 64)
    return dstack, smat


# packed const layout (one DMA): cols [0:256) dstack (parts 0:64 valid),
# [256:320) wt2, [320:384) smat, [384:388) pvec
CONST_W = 388


def _build():
    nc = bass.Bass("TRN2", target_bir_lowering=False, debug=False)
    xti = nc.dram_tensor("xti", [QL, BL * F_IN], DT, kind="ExternalInput").ap()
    consts = nc.dram_tensor("consts", [128, CONST_W], DT, kind="ExternalInput").ap()
    outd = nc.dram_tensor("out", [128, TH * QU], DT, kind="ExternalOutput").ap()

    F32R = mybir.dt.float32r
    Sq = mybir.ActivationFunctionType.Square
    HT = TH // 2  # 8 batch rows per pipeline half per partition-half

    with tile.TileContext(nc) as tc, ExitStack() as ctx:
        pool = ctx.enter_context(tc.tile_pool(name="main", bufs=1))
        psA = ctx.enter_context(tc.tile_pool(name="psA", bufs=3, space="PSUM"))
        psC = ctx.enter_context(tc.tile_pool(name="psC", bufs=2, space="PSUM"))

        # ---- PE warm-up: dummy matmul burst while input DMAs land -------
        # HAM un-throttles the PE only after ~3.4us of sustained activity;
        # burn the DMA-wait window so phase A runs at full clock.
        wsrc = pool.tile([64, 256], DT, tag="wsrc")
        nc.gpsimd.memset(wsrc[:], 1.0)
        wps = psA.tile([128, 4, 256], DT, tag="ya")
        for ww in range(6):
            nc.tensor.matmul(
                wps[:, ww % 4, 0:128],
                wsrc[:, 0:128],
                wsrc[:, 0:128],
                start=True,
                stop=True,
            )

        # ---- input loads: ds first, xti halves on the parallel DGE ------
        dsq = pool.tile([QL, 256], DT, tag="dsq")
        ds_sb = dsq[:]
        xti_sb = pool.tile([QL, BL * F_IN], DT, tag="xti")
        nc.scalar.dma_start(out=xti_sb[:, 0:1024].bitcast(mybir.dt.float32r), in_=xti[:, 0:1024].bitcast(mybir.dt.float32r))
        nc.sync.dma_start(
            out=dsq[:].bitcast(mybir.dt.float32r),
            in_=consts[0:64, 0:256].bitcast(mybir.dt.float32r),
        )
        cst = pool.tile([128, CONST_W - 256], DT, tag="cst")
        wt_sb = cst[:, 0:64]
        smat_sb = cst[:, 64:128]
        pvec_sb = cst[:, 128:132]
        nc.scalar.dma_start(out=xti_sb[:, 1024:2048].bitcast(mybir.dt.float32r), in_=xti[:, 1024:2048].bitcast(mybir.dt.float32r))
        nc.sync.dma_start(out=cst[:], in_=consts[:, 256:CONST_W])

        # ---- block-diagonal coefficient blocks ---------------------------
        # cp[:, g, :] = [[C_g, 0], [0, C_g]] with C_g a scaled power of W,
        # so each phase-C group is ONE K=128, M=128 matmul that contracts
        # the two batch partition-halves independently.
        # wt2 is W^T replicated on both partition halves: wt2[(q,k), j]=W[j,k]
        w2 = pool.tile([128, F_OUT], DT, tag="w2")
        nc.vector.tensor_mul(w2[:], wt_sb, wt_sb)
        w3 = pool.tile([128, F_OUT], DT, tag="w3")
        nc.vector.tensor_mul(w3[:], w2[:], wt_sb)
        cp = pool.tile([128, 6, 128], DT, tag="cp")
        cpr = cp[:].bitcast(mybir.dt.float32r)
        zsrc = bass.AP(
            tensor=smat_sb.tensor, offset=smat_sb.offset,
            ap=[smat_sb.ap[0], [0, 12], smat_sb.ap[1]],
        )
        nc.vector.tensor_scalar_mul(
            cp[:].rearrange("a g c -> a (g c)").bitcast(mybir.dt.float32r),
            zsrc, 0.0,
        )
        # with the dstack beta-folding (Y2'=-Y2, Y3'=Y3/3) every group's
        # coefficient block is just a half/third-scaled power of W
        blocks = [(0, wt_sb, -1.0), (1, w2[:], -0.5), (2, w3[:], -0.5),
                  (3, w2[:], -0.5), (4, w3[:], -0.5), (5, w3[:], -1.0 / 3.0)]
        for g, src, alpha in blocks:
            nc.vector.tensor_scalar_mul(
                cpr[0:64, g, 0:64], src[0:64, :], alpha)
            nc.gpsimd.tensor_scalar_mul(
                cpr[64:128, g, 64:128], src[64:128, :], alpha)

        # ---- expB[(q,j), l] = bq*(s-lamq)^2 + ba*|s-lama| ---------------
        sub = mybir.AluOpType.subtract
        tq = pool.tile([128, QU], DT, tag="tq")
        nc.vector.tensor_scalar(tq[:], smat_sb, pvec_sb[:, 0:1], None, op0=sub)
        tq2 = pool.tile([128, QU], DT, tag="tq2")
        nc.scalar.activation(tq2[:], tq[:], Sq)
        ta = pool.tile([128, QU], DT, tag="ta")
        nc.vector.tensor_scalar(ta[:], smat_sb, pvec_sb[:, 2:3], None, op0=sub)
        ta2 = pool.tile([128, QU], DT, tag="ta2")
        nc.scalar.activation(ta2[:], ta[:], mybir.ActivationFunctionType.Abs)
        eb1 = pool.tile([128, QU], DT, tag="eb1")
        nc.vector.tensor_scalar_mul(eb1[:], tq2[:], pvec_sb[:, 1:2])
        eb2 = pool.tile([128, QU], DT, tag="eb2")
        nc.vector.tensor_scalar_mul(eb2[:], ta2[:], pvec_sb[:, 3:4])
        ebs = pool.tile([128, QU], DT, tag="ebs")
        nc.vector.tensor_add(ebs[:], eb1[:], eb2[:])
        ebs_ap = ebs[:]
        ebs_q = bass.AP(
            tensor=ebs_ap.tensor,
            offset=ebs_ap.offset,
            ap=[ebs_ap.ap[0], [0, HT], ebs_ap.ap[1]],
        )

        # ---- pipelined halves: A -> evac -> products -> C -> epilogue ---
        ytil = pool.tile([128, 3, TH, QU], DT, tag="ytil")
        p11 = pool.tile([128, TH, QU], DT, tag="p11")
        p12 = pool.tile([128, TH, QU], DT, tag="p12")
        p111 = pool.tile([128, TH, QU], DT, tag="p111")
        lsb = pool.tile([128, TH, QU], DT, tag="lsb")
        esb = pool.tile([128, TH, QU], DT, tag="esb")
        sums = pool.tile([128, TH], DT, tag="sums")
        rsum = pool.tile([128, TH], DT, tag="rsum")
        outsb = pool.tile([128, TH, QU], DT, tag="outsb")
        outv = outd.rearrange("a (t l) -> a t l", l=QU)

        caccs = []
        for h in range(2):
            hs = bass.ts(h, HT)
            # phase A: 2 waves of 4 matmuls; evac wave0 on ACT, wave1 on DVE.
            # P11 is computed straight from the PSUM wave tiles so the cubic
            # product only waits on the evac, not a product ladder.
            for w in range(2):
                ya = psA.tile([128, 4, 256], DT, tag="ya")
                for j in range(4):
                    t = 8 * h + 4 * w + j
                    nc.tensor.matmul(
                        ya[:, j, :],
                        xti_sb[:, bass.ts(t, 128)].bitcast(F32R),
                        ds_sb.bitcast(F32R),
                        start=True,
                        stop=True,
                    )
                src = ya[:, :, 0 : 3 * QU].rearrange(
                    "a t (p l) -> a p t l", p=3, l=QU
                )
                ws = bass.ts(2 * h + w, 4)
                dst = ytil[:, :, ws, :]
                if w == 0:
                    nc.scalar.copy(out=dst.bitcast(mybir.dt.float32r), in_=src)
                else:
                    nc.vector.tensor_copy(dst.bitcast(mybir.dt.float32r), src)
                y1p = ya[:, :, 0:QU]
                nc.scalar.activation(
                    p11[:, ws, :].bitcast(mybir.dt.float32r), y1p, Sq
                )

            y1v = ytil[:, 0, hs, :]
            y2v = ytil[:, 1, hs, :]
            y3v = ytil[:, 2, hs, :]
            nc.gpsimd.tensor_mul(p12[:, hs, :].bitcast(mybir.dt.float32r), y1v, y2v)
            nc.vector.tensor_mul(p111[:, hs, :].bitcast(mybir.dt.float32r), p11[:, hs, :], y1v)

            # phase C: 6 block-diagonal groups, K=128, M=128, N=512
            cacc = psC.tile([128, HT * QU], DT, tag="cacc")
            caccs.append(cacc)
            groups = [
                (0, y1v), (1, y2v), (2, y3v),
                (3, p11[:, hs, :]), (4, p12[:, hs, :]), (5, p111[:, hs, :]),
            ]
            ng = len(groups)
            for g, (blk, z) in enumerate(groups):
                zf = z.rearrange("a t l -> a (t l)")
                nc.tensor.matmul(
                    cacc[:],
                    cp[:, blk, :].bitcast(F32R),
                    zf.bitcast(F32R),
                    start=(g == 0),
                    stop=(g == ng - 1),
                )

        # epilogues after both halves' compute so the scheduler never
        # starves h1's evac/products behind h0's epilogue
        for h in range(2):
            hs = bass.ts(h, HT)
            caccv = caccs[h][:].rearrange("a (t l) -> a t l", l=QU)
            nc.vector.tensor_sub(lsb[:, hs, :], caccv, ebs_q)
            nc.scalar.activation(
                esb[:, hs, :], lsb[:, hs, :], mybir.ActivationFunctionType.Exp
            )
            nc.vector.tensor_reduce(
                sums[:, hs], esb[:, hs, :], axis=mybir.AxisListType.X,
                op=mybir.AluOpType.add,
            )
            nc.vector.reciprocal(rsum[:, hs], sums[:, hs])
            rb = rsum[:, hs].to_broadcast((128, HT, QU))
            if h == 0:
                nc.gpsimd.tensor_mul(outsb[:, hs, :], esb[:, hs, :], rb)
                nc.sync.dma_start(out=outv[:, hs, :], in_=outsb[:, hs, :])
            else:
                nc.vector.tensor_mul(outsb[:, hs, :], esb[:, hs, :], rb)
                nc.scalar.dma_start(out=outv[:, hs, :], in_=outsb[:, hs, :])

    _split_waits(nc)
    return nc


def _prep_core_inputs(x, W, ba, bq, lama, lamq):
    """Host-side layout-only prep: shard, transpose, replicate, pack."""
    dstack, smat = _host_consts()
    consts = np.zeros((128, CONST_W), dtype=np.float32)
    consts[0:64, 0:256] = dstack
    consts[:, 256:320] = np.tile(W.T, (2, 1))
    consts[:, 320:384] = smat
    consts[:, 384:388] = np.tile(
        np.concatenate([lamq, bq, lama, ba], axis=1), (2, 1)
    )
    in_maps = []
    for c in range(NCORES):
        xc = x[c * BL : (c + 1) * BL]                  # (32, k, m)
        xt = xc.transpose(2, 0, 1)                     # (m, i, k)
        xt = xt.reshape(QL, 2, TH, F_IN).transpose(0, 2, 1, 3)  # (m, t, ih, k)
        xti = np.ascontiguousarray(xt.reshape(QL, BL * F_IN), dtype=np.float32)
        in_maps.append({"xti": xti, "consts": consts})
    return in_maps


def kernel(x, W, ba, bq, lama, lamq):
    if "nc" not in _CACHE:
        _CACHE["nc"] = _build()
    nc = _CACHE["nc"]
    in_maps = _prep_core_inputs(x, W, ba, bq, lama, lamq)
    res = run_bass_kernel_spmd(nc, in_maps, core_ids=list(range(NCORES)))
    outs = []
    for c in range(NCORES):
        o = res.results[c]["out"].reshape(2, F_OUT, TH, QU)   # (ih, j, t, l)
        o = o.transpose(0, 2, 1, 3).reshape(BL, F_OUT, QU)    # (i, j, l)
        outs.append(o)
    return np.ascontiguousarray(np.concatenate(outs, axis=0), dtype=np.float32)
